# revision 3
# baseline (speedup 1.0000x reference)
"""Chamfer distance loss kernel for 8x trn2 NeuronCores.

pred/target: [8, 4096, 3] f32. Output: scalar f32 (shape ()).

Strategy: data-parallel over batch (1 batch per core). Host prepares
augmented K=5 matrices so a single small-K matmul emits squared
pairwise distances directly into PSUM:
    D[n, m] = sum_k A[k, n] * B[k, m]
    A rows: [-2*px, -2*py, -2*pz, |p|^2, 1]
    B rows: [ tx,    ty,    tz,   1,    |t|^2]
On device, per [128, 2048] PSUM tile: row-min (reduce over free dim)
and a running elementwise col-min accumulator in SBUF. Col-min's
partition-axis reduction happens via PE transpose + strided reduce.
Host sums the per-core [128, 32] partial mins.
"""

import sys

import numpy as np

for _p in ("/opt/trn_rl_repo",):
    if _p not in sys.path:
        sys.path.insert(0, _p)

import concourse.bass as bass
import concourse.mybir as mybir
import concourse.tile as tile
from concourse import bacc, bass2jax
from concourse.masks import make_identity

B = 8
NPTS = 4096
K_AUG = 5
P = 128
N_TILES = NPTS // P  # 32
MG = 2048  # m-group width (4 PSUM banks)
N_GROUPS = NPTS // MG  # 2
MM_N = 512  # free dim per matmul (1 PSUM bank)
BIG = 3.0e38

_cached = {}


def build_nc(repeat=1):
    f32 = mybir.dt.float32
    nc = bacc.Bacc("TRN2", target_bir_lowering=False, debug=False, num_devices=B)

    a_dram = nc.dram_tensor("a", [K_AUG, NPTS], f32, kind="ExternalInput")
    b_dram = nc.dram_tensor("b", [K_AUG, NPTS], f32, kind="ExternalInput")
    rowpart_dram = nc.dram_tensor(
        "rowpart", [P, N_TILES * N_GROUPS], f32, kind="ExternalOutput"
    )
    colmins_dram = nc.dram_tensor("colmins", [P, N_TILES], f32, kind="ExternalOutput")

    with tile.TileContext(nc) as tc:
        with (
            tc.tile_pool(name="const", bufs=1) as cpool,
            tc.tile_pool(name="acc", bufs=1) as apool,
            tc.tile_pool(name="psum", bufs=2, space=bass.MemorySpace.PSUM) as ppool,
        ):
            a_sb = cpool.tile([K_AUG, NPTS], f32)
            b_sb = cpool.tile([K_AUG, NPTS], f32)
            ident = cpool.tile([P, P], f32)
            nc.sync.dma_start(a_sb[:], a_dram[:])
            nc.sync.dma_start(b_sb[:], b_dram[:])
            make_identity(nc, ident[:])

            cacc = apool.tile([P, NPTS], f32)  # running col-min accumulator
            rowpart_sb = apool.tile([P, N_TILES * N_GROUPS], f32)
            colmins_sb = apool.tile([P, N_TILES], f32)

            for _rep in range(repeat):
                nc.gpsimd.memset(cacc[:], BIG)

                for t in range(N_TILES):
                    lhsT = a_sb[:, t * P : (t + 1) * P]
                    for g in range(N_GROUPS):
                        pt = ppool.tile([P, MG], f32, tag="ptile")
                        for j in range(MG // MM_N):
                            off = g * MG + j * MM_N
                            nc.tensor.matmul(
                                pt[:, j * MM_N : (j + 1) * MM_N],
                                lhsT,
                                b_sb[:, off : off + MM_N],
                                start=True,
                                stop=True,
                            )
                        # dir-2: running elementwise col-min (in place)
                        nc.vector.tensor_tensor(
                            out=cacc[:, g * MG : (g + 1) * MG],
                            in0=pt[:],
                            in1=cacc[:, g * MG : (g + 1) * MG],
                            op=mybir.AluOpType.min,
                        )
                        # dir-1: row-min partial for this (t, g)
                        col = t * N_GROUPS + g
                        nc.vector.tensor_reduce(
                            out=rowpart_sb[:, col : col + 1],
                            in_=pt[:],
                            axis=mybir.AxisListType.X,
                            op=mybir.AluOpType.min,
                        )

                # col-min partition-axis reduction: transpose 128-wide chunks
                # with PE, then strided reduce (innermost axis only).
                for gg in range(N_TILES // 4):
                    tp = ppool.tile([P, 4, P], f32, tag="ptile")
                    for j in range(4):
                        c = gg * 4 + j
                        nc.tensor.transpose(
                            tp[:, j, :], cacc[:, c * P : (c + 1) * P], ident[:]
                        )
                    nc.vector.tensor_reduce(
                        out=colmins_sb[:, gg * 4 : (gg + 1) * 4],
                        in_=tp[:],
                        axis=mybir.AxisListType.X,
                        op=mybir.AluOpType.min,
                    )

                nc.sync.dma_start(rowpart_dram[:], rowpart_sb[:])
                nc.sync.dma_start(colmins_dram[:], colmins_sb[:])

    nc.compile()
    return nc


class Runner:
    """Caches the jitted shard_map executable across calls (the stock
    run_bass_kernel_spmd axon path rebuilds it per call, ~300 ms)."""

    def __init__(self, nc, n_cores):
        import jax
        from jax.experimental.shard_map import shard_map
        from jax.sharding import Mesh, PartitionSpec

        bass2jax.install_neuronx_cc_hook()
        self.nc = nc
        self.n_cores = n_cores
        partition_name = (
            nc.partition_id_tensor.name if nc.partition_id_tensor else None
        )
        in_names, out_names, out_avals, zero_outs = [], [], [], []
        for alloc in nc.m.functions[0].allocations:
            if not isinstance(alloc, mybir.MemoryLocationSet):
                continue
            name = alloc.memorylocations[0].name
            if alloc.kind == "ExternalInput":
                if name != partition_name:
                    in_names.append(name)
            elif alloc.kind == "ExternalOutput":
                shape = tuple(alloc.tensor_shape)
                dtype = mybir.dt.np(alloc.dtype)
                out_avals.append(jax.core.ShapedArray(shape, dtype))
                zero_outs.append(np.zeros(shape, dtype))
                out_names.append(name)
        self.in_names = list(in_names)
        self.out_names = out_names
        self.out_avals = out_avals
        self.zero_outs = zero_outs
        n_params = len(in_names)
        all_names = in_names + out_names
        if partition_name is not None:
            all_names = all_names + [partition_name]

        def _body(*args):
            operands = list(args)
            if partition_name is not None:
                operands.append(bass2jax.partition_id_tensor())
            outs = bass2jax._bass_exec_p.bind(
                *operands,
                out_avals=tuple(out_avals),
                in_names=tuple(all_names),
                out_names=tuple(out_names),
                lowering_input_output_aliases=(),
                sim_require_finite=True,
                sim_require_nnan=True,
                nc=nc,
            )
            return tuple(outs)

        devices = jax.devices()[:n_cores]
        mesh = Mesh(np.asarray(devices), ("core",))
        n_outs = len(out_names)
        self._sharded = jax.jit(
            shard_map(
                _body,
                mesh=mesh,
                in_specs=(PartitionSpec("core"),) * (n_params + n_outs),
                out_specs=(PartitionSpec("core"),) * n_outs,
                check_rep=False,
            ),
            donate_argnums=tuple(range(n_params, n_params + n_outs)),
            keep_unused=True,
        )

    def run_raw(self, in_maps):
        """Returns unblocked jax output arrays (call np.asarray to sync)."""
        n = self.n_cores
        concat_in = [
            np.concatenate([in_maps[c][name] for c in range(n)], axis=0)
            for name in self.in_names
        ]
        concat_zeros = [
            np.zeros((n * z.shape[0], *z.shape[1:]), z.dtype) for z in self.zero_outs
        ]
        return self._sharded(*concat_in, *concat_zeros)

    def __call__(self, in_maps):
        out_arrs = self.run_raw(in_maps)
        n = self.n_cores
        return [
            {
                name: np.asarray(out_arrs[i]).reshape(n, *self.out_avals[i].shape)[c]
                for i, name in enumerate(self.out_names)
            }
            for c in range(n)
        ]


def get_runner(repeat=1):
    if repeat not in _cached:
        _cached[repeat] = Runner(build_nc(repeat=repeat), B)
    return _cached[repeat]


def make_in_maps(pred, target):
    in_maps = []
    for c in range(B):
        p = np.ascontiguousarray(pred[c], dtype=np.float32)
        t = np.ascontiguousarray(target[c], dtype=np.float32)
        psq = (p * p).sum(axis=1)
        tsq = (t * t).sum(axis=1)
        a = np.empty((K_AUG, NPTS), dtype=np.float32)
        a[0:3] = -2.0 * p.T
        a[3] = psq
        a[4] = 1.0
        bm = np.empty((K_AUG, NPTS), dtype=np.float32)
        bm[0:3] = t.T
        bm[3] = 1.0
        bm[4] = tsq
        in_maps.append({"a": a, "b": bm})
    return in_maps


def finalize(results):
    total = 0.0
    for r in results:
        rowpart = r["rowpart"].reshape(P, N_TILES, N_GROUPS)
        rowmin = rowpart.min(axis=2)  # [128, 32] true row mins
        colmin = r["colmins"]  # [128, 32] true col mins
        total += rowmin.sum(dtype=np.float64) + colmin.sum(dtype=np.float64)
    return np.asarray(total / (B * NPTS), dtype=np.float32)


def kernel(pred, target):
    pred = np.asarray(pred)
    target = np.asarray(target)
    assert pred.shape == (B, NPTS, 3) and target.shape == (B, NPTS, 3)
    return finalize(get_runner()(make_in_maps(pred, target)))
